# revision 49
# baseline (speedup 1.0000x reference)
"""Causal self-attention (B=4, T=4096, D=1024, fp32) on 8 trn2 NeuronCores.

Sharding: 2 cores per batch. Within a batch, core h in {0,1} owns the
key blocks of parity h (128-wide blocks at global positions 2j+h). Each
core computes, for ALL queries of its batch, the unnormalized partial
attention output restricted to its own keys, already pushed through the
output projection, plus the partial softmax denominators:

    outT_h = W_o @ (sum_{k in parity h, k<=q} exp(s_qk) * v_k)^T
    denom_h[q] = sum_{k in parity h, k<=q} exp(s_qk)

Host merge: out[q] = (outT_0[:,q] + outT_1[:,q]) / (denom_0[q] + denom_1[q]).

Precision plan (fp8 where softmax washes the noise, bf16 where it can't):
  - Q/K projections and the score matmuls run in fp8e4 with DoubleRow
    (2x PE rate). Weights are pre-scaled by 32 on the host so q/k land
    in e4m3's normal range; the exp activation applies the combined
    1/(32*32*32) score scale.
  - The first q-tile (queries 0..TQ-1, where softmax averages few keys
    and sets the output absmax) instead uses a bf16 sidecar: every core
    projects those queries and its first two key blocks in bf16.
  - V projection, exp panel, attV and the output projection stay bf16.
  - Softmax denominators: fp32 pre-sum of the exp panel on the Vector
    engine + one fp32 ones-matmul per q-tile.

Layout: every DRAM tensor is host-swizzled to the SBUF tiling
([partition, blocks, cols], partition-major) so each DMA moves long
contiguous lines per partition (few descriptors, fast issue). Bulk
weight/x loads are issued from the (otherwise idle) GpSimd DGE queue;
fine-grained in-loop traffic stays on the Sync queue.

Q^T is deduplicated across the pair: each core projects only its own
contiguous half of the queries in fp8, then two chunked pair-wise
AllGathers (hidden under the K/V projection phase) distribute it.
"""

import sys

if "/opt/trn_rl_repo" not in sys.path:
    sys.path.insert(0, "/opt/trn_rl_repo")

import numpy as np
import ml_dtypes

BF16 = ml_dtypes.bfloat16
FP8 = ml_dtypes.float8_e4m3

D = 1024
P = 128          # partition / contraction block
DB = D // P      # 8 d-blocks
WS = 32.0        # fp8 weight pre-scale

_PROGRAM_CACHE = {}


def build_program(T, TQ):
    """Build + compile the single-core SPMD program. Returns the Bacc."""
    import concourse.mybir as mybir
    import concourse.tile as tile
    from concourse import bacc

    bf = mybir.dt.bfloat16
    f8 = mybir.dt.float8e4
    f32 = mybir.dt.float32
    DR = mybir.MatmulPerfMode.DoubleRow

    NT = T // TQ             # q-tiles per core (8)
    NM = TQ // 256           # diagonal (masked) key blocks per q-tile (2)
    TKV = T // 2             # parity keys per core (2048)
    NKB = TKV // P           # local key blocks (16)
    KV_TT = 512              # token tile for the projection phases
    NKVT = TKV // KV_TT      # 4
    TH = T // 2              # this core's query half (2048)
    NQT = TH // KV_TT        # q-projection token tiles (4)
    GC = TH // 2             # gather chunk width (1024)
    ITC = GC // KV_TT        # token tiles per gather chunk (2)
    ESC = 1.0 / (WS * WS * np.sqrt(np.float64(D)))  # fp8 exp scale
    ESC_BF = 1.0 / np.sqrt(np.float64(D))           # bf16 sidecar exp scale

    nc = bacc.Bacc("TRN2", target_bir_lowering=False, debug=False, num_devices=8)

    # All tensors are pre-swizzled on the host to partition-major layout.
    xT_q8 = nc.dram_tensor("xT_q8", [P, NQT, DB, KV_TT], f8,
                           kind="ExternalInput")
    xT_kv8 = nc.dram_tensor("xT_kv8", [P, NKVT, DB, KV_TT], f8,
                            kind="ExternalInput")
    xT_kv = nc.dram_tensor("xT_kv", [P, NKVT, DB, KV_TT], bf,
                           kind="ExternalInput")
    xq0_bf = nc.dram_tensor("xq0_bf", [P, DB, TQ], bf, kind="ExternalInput")
    w_q8 = nc.dram_tensor("w_q8", [P, DB, D], f8, kind="ExternalInput")
    w_k8 = nc.dram_tensor("w_k8", [P, DB, D], f8, kind="ExternalInput")
    w_qT = nc.dram_tensor("w_qT", [P, DB, D], bf, kind="ExternalInput")
    w_kT = nc.dram_tensor("w_kT", [P, DB, D], bf, kind="ExternalInput")
    w_vT = nc.dram_tensor("w_vT", [P, DB, D], bf, kind="ExternalInput")
    w_oT = nc.dram_tensor("w_oT", [P, DB, D], bf, kind="ExternalInput")
    mask = nc.dram_tensor("mask", [P, NM, TQ], bf, kind="ExternalInput")
    outT = nc.dram_tensor("outT", [P, NT, DB, TQ], bf, kind="ExternalOutput")
    denom = nc.dram_tensor("denom", [1, NT * TQ], f32, kind="ExternalOutput")

    with tile.TileContext(nc) as tc:
        with tc.tile_pool(name="res", bufs=1) as res, \
             tc.tile_pool(name="dram", bufs=1, space="DRAM") as dram:
            # Persistent SBUF
            kT8_sb = res.tile([P, DB, TKV], f8)      # fp8 K^T (d-major)
            kT0_sb = res.tile([P, DB, 2 * P], bf)    # bf16 K^T, key blocks 0,1
            v_sb = res.tile([P, NKB, D], bf)         # V (token-major)
            wq8_sb = res.tile([P, DB, D], f8)
            wk8_sb = res.tile([P, DB, D], f8)
            wv_sb = res.tile([P, DB, D], bf)
            wo_sb = res.tile([P, DB, D], bf)
            qT0_sb = res.tile([P, DB, TQ], bf)       # bf16 Q^T of q-tile 0
            mask_sb = res.tile([P, NM, TQ], bf)
            ones_sb = res.tile([P, 1], bf)
            nc.vector.memset(ones_sb[:], 1.0)

            # Pair-gathered fp8 Q^T, two column chunks. Chunk c holds local
            # token tiles [c*ITC, (c+1)*ITC) of BOTH ranks.
            qT_loc = [dram.tile([P, ITC, DB, KV_TT], f8, tag=f"qloc{c}",
                                name=f"qT_loc{c}") for c in range(2)]
            qT_ful = [dram.tile([2, P, ITC, DB, KV_TT], f8, tag=f"qful{c}",
                                name=f"qT_ful{c}") for c in range(2)]

            with tc.tile_pool(name="pproj", bufs=2) as pq_sb, \
                 tc.tile_pool(name="pq_ps", bufs=4, space="PSUM") as pq_ps:
                side = tc.alloc_tile_pool(name="side", bufs=1)
                # ---- Upfront bulk loads, ordered by need-time. Host
                # pre-swizzle makes each a long contiguous line/partition.
                xq8 = [pq_sb.tile([P, DB, KV_TT], f8, tag="xq8", bufs=4,
                                  name=f"xq8_t{it}") for it in range(NQT)]
                nc.sync.dma_start(xq8[0][:], xT_q8[:, 0])
                nc.sync.dma_start(wq8_sb[:], w_q8[:])
                nc.sync.dma_start(xq8[1][:], xT_q8[:, 1])
                xq0 = side.tile([P, DB, TQ], bf, tag="xq0")
                wq_bf = side.tile([P, DB, D], bf, tag="wqbf")
                wk_bf = side.tile([P, DB, D], bf, tag="wkbf")
                nc.sync.dma_start(xq0[:], xq0_bf[:])
                nc.sync.dma_start(wq_bf[:], w_qT[:])
                for it in range(2, NQT):
                    nc.sync.dma_start(xq8[it][:], xT_q8[:, it])
                xkv_bf0 = pq_sb.tile([P, DB, KV_TT], bf, tag="xkvbf", bufs=3)
                nc.sync.dma_start(xkv_bf0[:], xT_kv[:, 0])
                nc.sync.dma_start(wk_bf[:], w_kT[:])
                nc.sync.dma_start(wk8_sb[:], w_k8[:])
                nc.sync.dma_start(wv_sb[:], w_vT[:])
                nc.sync.dma_start(wo_sb[:], w_oT[:])
                nc.sync.dma_start(mask_sb[:], mask[:])

                def q8_proj(it):
                    qstage = pq_sb.tile([P, DB, KV_TT], f8, tag="qstage")
                    for do in range(DB):
                        qp = pq_ps.tile([P, KV_TT], f32, tag="qp")
                        for dp in range(DB // 2):
                            nc.tensor.matmul(
                                qp[:],
                                wq8_sb[:, 2 * dp:2 * dp + 2, do * P:(do + 1) * P],
                                xq8[it][:, 2 * dp:2 * dp + 2, :],
                                start=(dp == 0), stop=(dp == DB // 2 - 1),
                                perf_mode=DR)
                        nc.vector.tensor_copy(qstage[:, do, :], qp[:])
                    c = it // ITC
                    nc.sync.dma_start(qT_loc[c][:, it % ITC], qstage[:])
                    if it % ITC == ITC - 1:
                        nc.gpsimd.collective_compute(
                            "AllGather",
                            mybir.AluOpType.bypass,
                            replica_groups=[[0, 1], [2, 3], [4, 5], [6, 7]],
                            ins=[qT_loc[c][:]],
                            outs=[qT_ful[c][:]],
                        )

                # ---- Phase A0b(1st half): fp8 Q projection, gather chunk 0
                # first so the collective's latency overlaps the sidecars.
                for it in range(ITC):
                    q8_proj(it)

                # ---- Phase A0a: bf16 sidecar Q projection of q-tile 0 ----
                for do in range(DB):
                    qp = pq_ps.tile([P, TQ], f32, tag="qp")
                    for di in range(DB):
                        nc.tensor.matmul(
                            qp[:],
                            wq_bf[:, di, do * P:(do + 1) * P],
                            xq0[:, di, :],
                            start=(di == 0), stop=(di == DB - 1))
                    nc.vector.tensor_copy(qT0_sb[:, do, :], qp[:])

                # ---- Phase Aa: bf16 sidecar K projection, key blocks 0,1 ----
                for do in range(DB):
                    kp0 = pq_ps.tile([P, 2 * P], f32, tag="kp0", bufs=2)
                    for di in range(DB):
                        nc.tensor.matmul(
                            kp0[:],
                            wk_bf[:, di, do * P:(do + 1) * P],
                            xkv_bf0[:, di, 0:2 * P],
                            start=(di == 0), stop=(di == DB - 1))
                    nc.vector.tensor_copy(kT0_sb[:, do, :], kp0[:])
                side.release()

                # ---- Phase A0b(2nd half): fp8 Q projection, gather chunk 1
                for it in range(ITC, NQT):
                    q8_proj(it)

                # ---- Phase Ab: fp8 K + bf16 V projection of parity keys ----
                for tt in range(NKVT):
                    if tt == 0:
                        xkv_bf = xkv_bf0
                    else:
                        xkv_bf = pq_sb.tile([P, DB, KV_TT], bf, tag="xkvbf",
                                            bufs=3)
                        nc.sync.dma_start(xkv_bf[:], xT_kv[:, tt])
                    xkv8 = pq_sb.tile([P, DB, KV_TT], f8, tag="xkv8", bufs=2)
                    nc.sync.dma_start(xkv8[:], xT_kv8[:, tt])
                    # fp8 K^T
                    for do in range(DB):
                        kps = pq_ps.tile([P, KV_TT], f32, tag="qp")
                        for dp in range(DB // 2):
                            nc.tensor.matmul(
                                kps[:],
                                wk8_sb[:, 2 * dp:2 * dp + 2, do * P:(do + 1) * P],
                                xkv8[:, 2 * dp:2 * dp + 2, :],
                                start=(dp == 0), stop=(dp == DB // 2 - 1),
                                perf_mode=DR)
                        nc.vector.tensor_copy(
                            kT8_sb[:, do, tt * KV_TT:(tt + 1) * KV_TT], kps[:])
                    # bf16 V[tok, dout] += x^T[din, tok].T @ W_v^T[din, dout]
                    for tb in range(KV_TT // P):
                        for dh in range(D // 512):
                            vps = pq_ps.tile([P, 512], f32, tag="qp")
                            for di in range(DB):
                                nc.tensor.matmul(
                                    vps[:],
                                    xkv_bf[:, di, tb * P:(tb + 1) * P],
                                    wv_sb[:, di, dh * 512:(dh + 1) * 512],
                                    start=(di == 0), stop=(di == DB - 1))
                            nc.vector.tensor_copy(
                                v_sb[:, tt * (KV_TT // P) + tb,
                                     dh * 512:(dh + 1) * 512], vps[:])

            # ---- Phase B: per q-tile attention + output projection ----
            with tc.tile_pool(name="pb_sb", bufs=2) as pb_sb, \
                 tc.tile_pool(name="pb_pan", bufs=2) as pb_pan, \
                 tc.tile_pool(name="pb_dst", bufs=1) as pb_dst, \
                 tc.tile_pool(name="mm_ps", bufs=2, space="PSUM") as mm_ps, \
                 tc.tile_pool(name="s_ps", bufs=3, space="PSUM") as s_ps, \
                 tc.tile_pool(name="y_ps", bufs=3, space="PSUM") as y_ps:
                dstage = pb_dst.tile([1, NT * TQ], f32)
                pend_wo = None  # (yT tile, q-tile index) awaiting projection

                def emit_wo(yT, i, last=False):
                    ostage = pb_sb.tile([P, DB, TQ], bf, tag="ostage")
                    for do in range(DB):
                        ops = mm_ps.tile([P, TQ], f32, tag="mm")
                        for di in range(DB):
                            nc.tensor.matmul(
                                ops[:],
                                wo_sb[:, di, do * P:(do + 1) * P],
                                yT[:, di, :],
                                start=(di == 0), stop=(di == DB - 1))
                        nc.vector.tensor_copy(ostage[:, do, :], ops[:])
                        if last:
                            nc.sync.dma_start(outT[:, i, do], ostage[:, do, :])
                    if not last:
                        nc.sync.dma_start(outT[:, i], ostage[:])

                for i in range(NT):
                    nkb = (i + 1) * NM  # local key blocks for this q-tile
                    q0 = i * TQ

                    if i > 0:
                        ho = i // (NT // 2)
                        lt = i % NQT          # local token tile in the half
                        c = lt // ITC
                        qT = pb_sb.tile([P, DB, TQ], f8, tag="qT")
                        nc.sync.dma_start(qT[:], qT_ful[c][ho, :, lt % ITC])

                    # S^T blocks -> exp -> (mask) -> panel; fp32 pre-sum of
                    # the denominator on the Vector engine.
                    panel = pb_pan.tile([P, NT * NM, TQ], bf, tag="panel")
                    dacc = pb_sb.tile([P, TQ], f32, tag="dacc")
                    for j in range(nkb):
                        sps = s_ps.tile([P, TQ], f32, tag="s")
                        if i == 0:
                            for di in range(DB):
                                nc.tensor.matmul(
                                    sps[:],
                                    kT0_sb[:, di, j * P:(j + 1) * P],
                                    qT0_sb[:, di, :],
                                    start=(di == 0), stop=(di == DB - 1))
                            esc = ESC_BF
                        else:
                            for dp in range(DB // 2):
                                nc.tensor.matmul(
                                    sps[:],
                                    kT8_sb[:, 2 * dp:2 * dp + 2,
                                           j * P:(j + 1) * P],
                                    qT[:, 2 * dp:2 * dp + 2, :],
                                    start=(dp == 0), stop=(dp == DB // 2 - 1),
                                    perf_mode=DR)
                            esc = ESC
                        nc.scalar.activation(
                            panel[:, j, :], sps[:],
                            mybir.ActivationFunctionType.Exp, scale=esc)
                        if j >= nkb - NM:
                            m = j - (nkb - NM)
                            nc.vector.tensor_mul(
                                out=panel[:, j, :], in0=panel[:, j, :],
                                in1=mask_sb[:, m, :])
                        # denominator pre-sum on GpSimd so the Vector FIFO
                        # stays free for the latency-critical PSUM casts
                        if j == 0:
                            nc.gpsimd.tensor_copy(dacc[:], panel[:, 0, :])
                        else:
                            nc.gpsimd.tensor_add(
                                out=dacc[:], in0=dacc[:], in1=panel[:, j, :])

                    # y^T[dout, q] += V[k, dout].T @ expS^T[k, q]
                    yT = pb_sb.tile([P, DB, TQ], bf, tag="yT")
                    for do in range(DB):
                        yps = y_ps.tile([P, TQ], f32, tag="y")
                        for j in range(nkb):
                            nc.tensor.matmul(
                                yps[:],
                                v_sb[:, j, do * P:(do + 1) * P],
                                panel[:, j, :],
                                start=(j == 0), stop=(j == nkb - 1))
                        nc.vector.tensor_copy(yT[:, do, :], yps[:])

                    # denominator: single bf16 partition-reduction matmul.
                    # dacc's bf16 rounding is independent across the 128
                    # partitions, so it averages out in the reduction.
                    # (borrows a row of a y-pool PSUM bank for ~1us)
                    dacc_bf = pb_sb.tile([P, TQ], bf, tag="daccbf")
                    nc.vector.tensor_copy(dacc_bf[:], dacc[:])
                    dps = y_ps.tile([P, TQ], f32, tag="y", name=f"dps_{i}")
                    nc.tensor.matmul(dps[0:1, :], ones_sb[:], dacc_bf[:],
                                     start=True, stop=True)
                    nc.vector.tensor_copy(dstage[0:1, q0:q0 + TQ], dps[0:1, :])

                    # output projection of the PREVIOUS tile (pipelined so
                    # its yT casts hide under this tile's matmuls)
                    if pend_wo is not None:
                        emit_wo(*pend_wo)
                    pend_wo = (yT, i)

                emit_wo(*pend_wo, last=True)
                nc.sync.dma_start(denom[:], dstage[:])

    nc.compile()
    return nc


def _swz_w(wT, dtype):
    """[D, D] row-major (d_in, f) -> [P, DB, D] partition-major."""
    return np.ascontiguousarray(
        wT.reshape(DB, P, D).transpose(1, 0, 2)).astype(dtype)


def _swz_x(xT, tile_w, dtype):
    """[D, T'] -> [P, nt, DB, tile_w] partition-major, token-tiled."""
    nt = xT.shape[1] // tile_w
    return np.ascontiguousarray(
        xT.reshape(DB, P, nt, tile_w).transpose(1, 2, 0, 3)).astype(dtype)


def _prepare_core_inputs(x, W_q, W_k, W_v, W_o, T, TQ):
    """Host-side shard prep. Returns list of 8 in_maps (bf16/fp8 ndarrays)."""
    B = x.shape[0]
    KV_TT = 512

    def clip8(a):
        return np.clip(a, -240.0, 240.0)

    w_qT = _swz_w(W_q.T, BF16)
    w_kT = _swz_w(W_k.T, BF16)
    w_vT = _swz_w(W_v.T, BF16)
    w_oT = _swz_w(W_o.T, BF16)
    w_q8 = _swz_w(clip8(W_q.T * WS), FP8)
    w_k8 = _swz_w(clip8(W_k.T * WS), FP8)

    # Diagonal masks per parity: mask[m][k, q] = 1 if k + 256*m + 128*h <= q
    NM = TQ // 256
    k_idx = np.arange(P)[:, None, None]
    m_idx = np.arange(NM)[None, :, None]
    q_idx = np.arange(TQ)[None, None, :]
    masks = [
        (k_idx + 256 * m_idx + P * h <= q_idx).astype(np.float32).astype(BF16)
        for h in (0, 1)
    ]

    in_maps = []
    for b in range(B):
        xb = x[b]                                   # [T, D] fp32
        xT = np.ascontiguousarray(xb.T)             # [D, T] fp32
        xT8 = clip8(xT)
        # parity gather of 128-wide key blocks
        xblk = xT.reshape(D, T // (2 * P), 2, P)
        xq0_bf = _swz_x(xT[:, 0:TQ], TQ, BF16)[:, 0]
        for h in (0, 1):
            xkv = xblk[:, :, h, :].reshape(D, T // 2)
            in_maps.append({
                "xT_q8": _swz_x(xT8[:, h * (T // 2):(h + 1) * (T // 2)],
                                KV_TT, FP8),
                "xT_kv8": _swz_x(clip8(xkv), KV_TT, FP8),
                "xT_kv": _swz_x(xkv, KV_TT, BF16),
                "xq0_bf": xq0_bf,
                "w_q8": w_q8, "w_k8": w_k8,
                "w_qT": w_qT, "w_kT": w_kT, "w_vT": w_vT, "w_oT": w_oT,
                "mask": masks[h],
            })
    return in_maps


def _merge(results, B, T):
    """Host merge: (out0+out1)/(d0+d1) per batch, back to [B, T, D] fp32."""
    out = np.empty((B, T, D), dtype=np.float32)
    for b in range(B):
        # outT is [P, NT, DB, TQ] partition-major; unswizzle to [D, T]
        def unswz(a):
            pi, nt, db, tq = a.shape
            return a.astype(np.float32).transpose(2, 0, 1, 3).reshape(
                D, nt * tq)
        o0 = unswz(results[2 * b]["outT"])
        o1 = unswz(results[2 * b + 1]["outT"])
        d0 = results[2 * b]["denom"].reshape(T)
        d1 = results[2 * b + 1]["denom"].reshape(T)
        out[b] = ((o0 + o1) / (d0 + d1)[None, :]).T
    return out


def kernel(x, W_q, W_k, W_v, W_o):
    from concourse.bass_utils import run_bass_kernel_spmd

    x = np.asarray(x)
    B, T, d = x.shape
    assert d == D
    TQ = 512

    key = (T, TQ)
    if key not in _PROGRAM_CACHE:
        _PROGRAM_CACHE[key] = build_program(T, TQ)
    nc = _PROGRAM_CACHE[key]

    in_maps = _prepare_core_inputs(
        np.asarray(x, np.float32), np.asarray(W_q, np.float32),
        np.asarray(W_k, np.float32), np.asarray(W_v, np.float32),
        np.asarray(W_o, np.float32), T, TQ)
    res = run_bass_kernel_spmd(nc, in_maps, list(range(2 * B)))
    return _merge(res.results, B, T)


# revision 50
# speedup vs baseline: 1.0182x; 1.0182x over previous
"""Causal self-attention (B=4, T=4096, D=1024, fp32) on 8 trn2 NeuronCores.

Sharding: 2 cores per batch. Within a batch, core h in {0,1} owns the
key blocks of parity h (128-wide blocks at global positions 2j+h). Each
core computes, for ALL queries of its batch, the unnormalized partial
attention output restricted to its own keys, already pushed through the
output projection, plus the partial softmax denominators:

    outT_h = W_o @ (sum_{k in parity h, k<=q} exp(s_qk) * v_k)^T
    denom_h[q] = sum_{k in parity h, k<=q} exp(s_qk)

Host merge: out[q] = (outT_0[:,q] + outT_1[:,q]) / (denom_0[q] + denom_1[q]).

Precision plan (fp8 where softmax washes the noise, bf16 where it can't):
  - Q/K projections and the score matmuls run in fp8e4 with DoubleRow
    (2x PE rate). Weights are pre-scaled by 32 on the host so q/k land
    in e4m3's normal range; the exp activation applies the combined
    1/(32*32*32) score scale.
  - The first q-tile (queries 0..TQ-1, where softmax averages few keys
    and sets the output absmax) instead uses a bf16 sidecar: every core
    projects those queries and its first two key blocks in bf16.
  - V projection, exp panel, attV and the output projection stay bf16.
  - Softmax denominators: fp32 pre-sum of the exp panel on the Vector
    engine + one fp32 ones-matmul per q-tile.

Layout: every DRAM tensor is host-swizzled to the SBUF tiling
([partition, blocks, cols], partition-major) so each DMA moves long
contiguous lines per partition (few descriptors, fast issue). Bulk
weight/x loads are issued from the (otherwise idle) GpSimd DGE queue;
fine-grained in-loop traffic stays on the Sync queue.

Q^T is deduplicated across the pair: each core projects only its own
contiguous half of the queries in fp8, then two chunked pair-wise
AllGathers (hidden under the K/V projection phase) distribute it.
"""

import sys

if "/opt/trn_rl_repo" not in sys.path:
    sys.path.insert(0, "/opt/trn_rl_repo")

import numpy as np
import ml_dtypes

BF16 = ml_dtypes.bfloat16
FP8 = ml_dtypes.float8_e4m3

D = 1024
P = 128          # partition / contraction block
DB = D // P      # 8 d-blocks
WS = 32.0        # fp8 weight pre-scale

_PROGRAM_CACHE = {}


def build_program(T, TQ):
    """Build + compile the single-core SPMD program. Returns the Bacc."""
    import concourse.mybir as mybir
    import concourse.tile as tile
    from concourse import bacc

    bf = mybir.dt.bfloat16
    f8 = mybir.dt.float8e4
    f32 = mybir.dt.float32
    DR = mybir.MatmulPerfMode.DoubleRow

    NT = T // TQ             # q-tiles per core (8)
    NM = TQ // 256           # diagonal (masked) key blocks per q-tile (2)
    TKV = T // 2             # parity keys per core (2048)
    NKB = TKV // P           # local key blocks (16)
    KV_TT = 512              # token tile for the projection phases
    NKVT = TKV // KV_TT      # 4
    TH = T // 2              # this core's query half (2048)
    NQT = TH // KV_TT        # q-projection token tiles (4)
    GC = TH // 2             # gather chunk width (1024)
    ITC = GC // KV_TT        # token tiles per gather chunk (2)
    ESC = 1.0 / (WS * WS * np.sqrt(np.float64(D)))  # fp8 exp scale
    ESC_BF = 1.0 / np.sqrt(np.float64(D))           # bf16 sidecar exp scale

    nc = bacc.Bacc("TRN2", target_bir_lowering=False, debug=False, num_devices=8)

    # All tensors are pre-swizzled on the host to partition-major layout.
    xT_q8 = nc.dram_tensor("xT_q8", [P, NQT, DB, KV_TT], f8,
                           kind="ExternalInput")
    xT_kv8 = nc.dram_tensor("xT_kv8", [P, NKVT, DB, KV_TT], f8,
                            kind="ExternalInput")
    xT_kv = nc.dram_tensor("xT_kv", [P, NKVT, DB, KV_TT], bf,
                           kind="ExternalInput")
    xq0_bf = nc.dram_tensor("xq0_bf", [P, DB, TQ], bf, kind="ExternalInput")
    w_q8 = nc.dram_tensor("w_q8", [P, DB, D], f8, kind="ExternalInput")
    w_k8 = nc.dram_tensor("w_k8", [P, DB, D], f8, kind="ExternalInput")
    w_qT = nc.dram_tensor("w_qT", [P, DB, D], bf, kind="ExternalInput")
    w_kT = nc.dram_tensor("w_kT", [P, DB, D], bf, kind="ExternalInput")
    w_vT = nc.dram_tensor("w_vT", [P, DB, D], bf, kind="ExternalInput")
    w_oT = nc.dram_tensor("w_oT", [P, DB, D], bf, kind="ExternalInput")
    mask = nc.dram_tensor("mask", [P, NM, TQ], bf, kind="ExternalInput")
    outT = nc.dram_tensor("outT", [P, NT, DB, TQ], bf, kind="ExternalOutput")
    denom = nc.dram_tensor("denom", [1, NT * TQ], f32, kind="ExternalOutput")

    with tile.TileContext(nc) as tc:
        with tc.tile_pool(name="res", bufs=1) as res, \
             tc.tile_pool(name="dram", bufs=1, space="DRAM") as dram:
            # Persistent SBUF
            kT8_sb = res.tile([P, DB, TKV], f8)      # fp8 K^T (d-major)
            kT0_sb = res.tile([P, DB, 2 * P], bf)    # bf16 K^T, key blocks 0,1
            v_sb = res.tile([P, NKB, D], bf)         # V (token-major)
            wq8_sb = res.tile([P, DB, D], f8)
            wk8_sb = res.tile([P, DB, D], f8)
            wv_sb = res.tile([P, DB, D], bf)
            wo_sb = res.tile([P, DB, D], bf)
            qT0_sb = res.tile([P, DB, TQ], bf)       # bf16 Q^T of q-tile 0
            mask_sb = res.tile([P, NM, TQ], bf)
            ones_sb = res.tile([P, 1], bf)
            nc.vector.memset(ones_sb[:], 1.0)

            # Pair-gathered fp8 Q^T, two column chunks. Chunk c holds local
            # token tiles [c*ITC, (c+1)*ITC) of BOTH ranks.
            qT_loc = [dram.tile([P, ITC, DB, KV_TT], f8, tag=f"qloc{c}",
                                name=f"qT_loc{c}") for c in range(2)]
            qT_ful = [dram.tile([2, P, ITC, DB, KV_TT], f8, tag=f"qful{c}",
                                name=f"qT_ful{c}") for c in range(2)]

            with tc.tile_pool(name="pproj", bufs=2) as pq_sb, \
                 tc.tile_pool(name="pq_ps", bufs=4, space="PSUM") as pq_ps:
                side = tc.alloc_tile_pool(name="side", bufs=1)
                # ---- Upfront bulk loads, ordered by need-time. Host
                # pre-swizzle makes each a long contiguous line/partition.
                xq8 = [pq_sb.tile([P, DB, KV_TT], f8, tag="xq8", bufs=4,
                                  name=f"xq8_t{it}") for it in range(NQT)]
                nc.sync.dma_start(xq8[0][:], xT_q8[:, 0])
                nc.sync.dma_start(wq8_sb[:], w_q8[:])
                nc.sync.dma_start(xq8[1][:], xT_q8[:, 1])
                xq0 = side.tile([P, DB, TQ], bf, tag="xq0")
                wq_bf = side.tile([P, DB, D], bf, tag="wqbf")
                wk_bf = side.tile([P, DB, D], bf, tag="wkbf")
                nc.sync.dma_start(xq0[:], xq0_bf[:])
                nc.sync.dma_start(wq_bf[:], w_qT[:])
                for it in range(2, NQT):
                    nc.sync.dma_start(xq8[it][:], xT_q8[:, it])
                xkv_bf0 = pq_sb.tile([P, DB, KV_TT], bf, tag="xkvbf", bufs=3)
                nc.sync.dma_start(xkv_bf0[:], xT_kv[:, 0])
                nc.sync.dma_start(wk_bf[:], w_kT[:])
                nc.sync.dma_start(wk8_sb[:], w_k8[:])
                nc.sync.dma_start(wv_sb[:], w_vT[:])
                nc.sync.dma_start(wo_sb[:], w_oT[:])
                nc.sync.dma_start(mask_sb[:], mask[:])

                def q8_proj(it):
                    qstage = pq_sb.tile([P, DB, KV_TT], f8, tag="qstage")
                    for do in range(DB):
                        qp = pq_ps.tile([P, KV_TT], f32, tag="qp")
                        for dp in range(DB // 2):
                            nc.tensor.matmul(
                                qp[:],
                                wq8_sb[:, 2 * dp:2 * dp + 2, do * P:(do + 1) * P],
                                xq8[it][:, 2 * dp:2 * dp + 2, :],
                                start=(dp == 0), stop=(dp == DB // 2 - 1),
                                perf_mode=DR)
                        nc.vector.tensor_copy(qstage[:, do, :], qp[:])
                    c = it // ITC
                    nc.sync.dma_start(qT_loc[c][:, it % ITC], qstage[:])
                    if it % ITC == ITC - 1:
                        nc.gpsimd.collective_compute(
                            "AllGather",
                            mybir.AluOpType.bypass,
                            replica_groups=[[0, 1], [2, 3], [4, 5], [6, 7]],
                            ins=[qT_loc[c][:]],
                            outs=[qT_ful[c][:]],
                        )

                # ---- Phase A0b(1st half): fp8 Q projection, gather chunk 0
                # first so the collective's latency overlaps the sidecars.
                for it in range(ITC):
                    q8_proj(it)

                # ---- Phase A0a: bf16 sidecar Q projection of q-tile 0 ----
                for do in range(DB):
                    qp = pq_ps.tile([P, TQ], f32, tag="qp")
                    for di in range(DB):
                        nc.tensor.matmul(
                            qp[:],
                            wq_bf[:, di, do * P:(do + 1) * P],
                            xq0[:, di, :],
                            start=(di == 0), stop=(di == DB - 1))
                    nc.vector.tensor_copy(qT0_sb[:, do, :], qp[:])

                # ---- Phase Aa: bf16 sidecar K projection, key blocks 0,1 ----
                for do in range(DB):
                    kp0 = pq_ps.tile([P, 2 * P], f32, tag="kp0", bufs=2)
                    for di in range(DB):
                        nc.tensor.matmul(
                            kp0[:],
                            wk_bf[:, di, do * P:(do + 1) * P],
                            xkv_bf0[:, di, 0:2 * P],
                            start=(di == 0), stop=(di == DB - 1))
                    nc.vector.tensor_copy(kT0_sb[:, do, :], kp0[:])
                side.release()

                # ---- Phase A0b(2nd half): fp8 Q projection, gather chunk 1
                for it in range(ITC, NQT):
                    q8_proj(it)

                # ---- Phase Ab: fp8 K + bf16 V projection of parity keys ----
                for tt in range(NKVT):
                    if tt == 0:
                        xkv_bf = xkv_bf0
                    else:
                        xkv_bf = pq_sb.tile([P, DB, KV_TT], bf, tag="xkvbf",
                                            bufs=3)
                        nc.sync.dma_start(xkv_bf[:], xT_kv[:, tt])
                    xkv8 = pq_sb.tile([P, DB, KV_TT], f8, tag="xkv8", bufs=2)
                    nc.sync.dma_start(xkv8[:], xT_kv8[:, tt])
                    # fp8 K^T
                    for do in range(DB):
                        kps = pq_ps.tile([P, KV_TT], f32, tag="qp")
                        for dp in range(DB // 2):
                            nc.tensor.matmul(
                                kps[:],
                                wk8_sb[:, 2 * dp:2 * dp + 2, do * P:(do + 1) * P],
                                xkv8[:, 2 * dp:2 * dp + 2, :],
                                start=(dp == 0), stop=(dp == DB // 2 - 1),
                                perf_mode=DR)
                        nc.vector.tensor_copy(
                            kT8_sb[:, do, tt * KV_TT:(tt + 1) * KV_TT], kps[:])
                    # bf16 V[tok, dout] += x^T[din, tok].T @ W_v^T[din, dout]
                    for tb in range(KV_TT // P):
                        for dh in range(D // 512):
                            vps = pq_ps.tile([P, 512], f32, tag="qp")
                            for di in range(DB):
                                nc.tensor.matmul(
                                    vps[:],
                                    xkv_bf[:, di, tb * P:(tb + 1) * P],
                                    wv_sb[:, di, dh * 512:(dh + 1) * 512],
                                    start=(di == 0), stop=(di == DB - 1))
                            nc.vector.tensor_copy(
                                v_sb[:, tt * (KV_TT // P) + tb,
                                     dh * 512:(dh + 1) * 512], vps[:])

            # ---- Phase B: per q-tile attention + output projection ----
            with tc.tile_pool(name="pb_sb", bufs=2) as pb_sb, \
                 tc.tile_pool(name="pb_pan", bufs=2) as pb_pan, \
                 tc.tile_pool(name="pb_dst", bufs=1) as pb_dst, \
                 tc.tile_pool(name="mm_ps", bufs=2, space="PSUM") as mm_ps, \
                 tc.tile_pool(name="s_ps", bufs=3, space="PSUM") as s_ps, \
                 tc.tile_pool(name="y_ps", bufs=3, space="PSUM") as y_ps:
                dstage = pb_dst.tile([1, NT * TQ], f32)
                pend_wo = None  # (yT tile, q-tile index) awaiting projection

                def emit_wo(yT, i, last=False):
                    ostage = pb_sb.tile([P, DB, TQ], bf, tag="ostage")
                    for do in range(DB):
                        ops = mm_ps.tile([P, TQ], f32, tag="mm")
                        for di in range(DB):
                            nc.tensor.matmul(
                                ops[:],
                                wo_sb[:, di, do * P:(do + 1) * P],
                                yT[:, di, :],
                                start=(di == 0), stop=(di == DB - 1))
                        nc.vector.tensor_copy(ostage[:, do, :], ops[:])
                        if last:
                            nc.sync.dma_start(outT[:, i, do], ostage[:, do, :])
                    if not last:
                        nc.sync.dma_start(outT[:, i], ostage[:])

                for i in range(NT):
                    nkb = (i + 1) * NM  # local key blocks for this q-tile
                    q0 = i * TQ

                    if i > 0:
                        ho = i // (NT // 2)
                        lt = i % NQT          # local token tile in the half
                        c = lt // ITC
                        qT = pb_sb.tile([P, DB, TQ], f8, tag="qT")
                        nc.sync.dma_start(qT[:], qT_ful[c][ho, :, lt % ITC])

                    # S^T blocks -> exp -> (mask) -> panel; fp32 pre-sum of
                    # the denominator on the Vector engine.
                    panel = pb_pan.tile([P, NT * NM, TQ], bf, tag="panel")
                    dacc = pb_sb.tile([P, TQ], f32, tag="dacc")
                    for j in range(nkb):
                        sps = s_ps.tile([P, TQ], f32, tag="s")
                        if i == 0:
                            for di in range(DB):
                                nc.tensor.matmul(
                                    sps[:],
                                    kT0_sb[:, di, j * P:(j + 1) * P],
                                    qT0_sb[:, di, :],
                                    start=(di == 0), stop=(di == DB - 1))
                            esc = ESC_BF
                        else:
                            for dp in range(DB // 2):
                                nc.tensor.matmul(
                                    sps[:],
                                    kT8_sb[:, 2 * dp:2 * dp + 2,
                                           j * P:(j + 1) * P],
                                    qT[:, 2 * dp:2 * dp + 2, :],
                                    start=(dp == 0), stop=(dp == DB // 2 - 1),
                                    perf_mode=DR)
                            esc = ESC
                        nc.scalar.activation(
                            panel[:, j, :], sps[:],
                            mybir.ActivationFunctionType.Exp, scale=esc)
                        if j >= nkb - NM:
                            m = j - (nkb - NM)
                            nc.vector.tensor_mul(
                                out=panel[:, j, :], in0=panel[:, j, :],
                                in1=mask_sb[:, m, :])
                        if j == 0:
                            nc.vector.tensor_copy(dacc[:], panel[:, 0, :])
                        else:
                            nc.vector.tensor_add(
                                out=dacc[:], in0=dacc[:], in1=panel[:, j, :])

                    # y^T[dout, q] += V[k, dout].T @ expS^T[k, q]
                    yT = pb_sb.tile([P, DB, TQ], bf, tag="yT")
                    for do in range(DB):
                        yps = y_ps.tile([P, TQ], f32, tag="y")
                        for j in range(nkb):
                            nc.tensor.matmul(
                                yps[:],
                                v_sb[:, j, do * P:(do + 1) * P],
                                panel[:, j, :],
                                start=(j == 0), stop=(j == nkb - 1))
                        nc.vector.tensor_copy(yT[:, do, :], yps[:])

                    # denominator: single bf16 partition-reduction matmul.
                    # dacc's bf16 rounding is independent across the 128
                    # partitions, so it averages out in the reduction.
                    # (borrows a row of a y-pool PSUM bank for ~1us)
                    dacc_bf = pb_sb.tile([P, TQ], bf, tag="daccbf")
                    nc.vector.tensor_copy(dacc_bf[:], dacc[:])
                    dps = y_ps.tile([P, TQ], f32, tag="y", name=f"dps_{i}")
                    nc.tensor.matmul(dps[0:1, :], ones_sb[:], dacc_bf[:],
                                     start=True, stop=True)
                    nc.vector.tensor_copy(dstage[0:1, q0:q0 + TQ], dps[0:1, :])

                    # output projection of the PREVIOUS tile (pipelined so
                    # its yT casts hide under this tile's matmuls)
                    if pend_wo is not None:
                        emit_wo(*pend_wo)
                    pend_wo = (yT, i)

                emit_wo(*pend_wo, last=True)
                nc.sync.dma_start(denom[:], dstage[:])

    nc.compile()
    return nc


def _swz_w(wT, dtype):
    """[D, D] row-major (d_in, f) -> [P, DB, D] partition-major."""
    return np.ascontiguousarray(
        wT.reshape(DB, P, D).transpose(1, 0, 2)).astype(dtype)


def _swz_x(xT, tile_w, dtype):
    """[D, T'] -> [P, nt, DB, tile_w] partition-major, token-tiled."""
    nt = xT.shape[1] // tile_w
    return np.ascontiguousarray(
        xT.reshape(DB, P, nt, tile_w).transpose(1, 2, 0, 3)).astype(dtype)


def _prepare_core_inputs(x, W_q, W_k, W_v, W_o, T, TQ):
    """Host-side shard prep. Returns list of 8 in_maps (bf16/fp8 ndarrays)."""
    B = x.shape[0]
    KV_TT = 512

    def clip8(a):
        return np.clip(a, -240.0, 240.0)

    w_qT = _swz_w(W_q.T, BF16)
    w_kT = _swz_w(W_k.T, BF16)
    w_vT = _swz_w(W_v.T, BF16)
    w_oT = _swz_w(W_o.T, BF16)
    w_q8 = _swz_w(clip8(W_q.T * WS), FP8)
    w_k8 = _swz_w(clip8(W_k.T * WS), FP8)

    # Diagonal masks per parity: mask[m][k, q] = 1 if k + 256*m + 128*h <= q
    NM = TQ // 256
    k_idx = np.arange(P)[:, None, None]
    m_idx = np.arange(NM)[None, :, None]
    q_idx = np.arange(TQ)[None, None, :]
    masks = [
        (k_idx + 256 * m_idx + P * h <= q_idx).astype(np.float32).astype(BF16)
        for h in (0, 1)
    ]

    in_maps = []
    for b in range(B):
        xb = x[b]                                   # [T, D] fp32
        xT = np.ascontiguousarray(xb.T)             # [D, T] fp32
        xT8 = clip8(xT)
        # parity gather of 128-wide key blocks
        xblk = xT.reshape(D, T // (2 * P), 2, P)
        xq0_bf = _swz_x(xT[:, 0:TQ], TQ, BF16)[:, 0]
        for h in (0, 1):
            xkv = xblk[:, :, h, :].reshape(D, T // 2)
            in_maps.append({
                "xT_q8": _swz_x(xT8[:, h * (T // 2):(h + 1) * (T // 2)],
                                KV_TT, FP8),
                "xT_kv8": _swz_x(clip8(xkv), KV_TT, FP8),
                "xT_kv": _swz_x(xkv, KV_TT, BF16),
                "xq0_bf": xq0_bf,
                "w_q8": w_q8, "w_k8": w_k8,
                "w_qT": w_qT, "w_kT": w_kT, "w_vT": w_vT, "w_oT": w_oT,
                "mask": masks[h],
            })
    return in_maps


def _merge(results, B, T):
    """Host merge: (out0+out1)/(d0+d1) per batch, back to [B, T, D] fp32."""
    out = np.empty((B, T, D), dtype=np.float32)
    for b in range(B):
        # outT is [P, NT, DB, TQ] partition-major; unswizzle to [D, T]
        def unswz(a):
            pi, nt, db, tq = a.shape
            return a.astype(np.float32).transpose(2, 0, 1, 3).reshape(
                D, nt * tq)
        o0 = unswz(results[2 * b]["outT"])
        o1 = unswz(results[2 * b + 1]["outT"])
        d0 = results[2 * b]["denom"].reshape(T)
        d1 = results[2 * b + 1]["denom"].reshape(T)
        out[b] = ((o0 + o1) / (d0 + d1)[None, :]).T
    return out


def kernel(x, W_q, W_k, W_v, W_o):
    from concourse.bass_utils import run_bass_kernel_spmd

    x = np.asarray(x)
    B, T, d = x.shape
    assert d == D
    TQ = 512

    key = (T, TQ)
    if key not in _PROGRAM_CACHE:
        _PROGRAM_CACHE[key] = build_program(T, TQ)
    nc = _PROGRAM_CACHE[key]

    in_maps = _prepare_core_inputs(
        np.asarray(x, np.float32), np.asarray(W_q, np.float32),
        np.asarray(W_k, np.float32), np.asarray(W_v, np.float32),
        np.asarray(W_o, np.float32), T, TQ)
    res = run_bass_kernel_spmd(nc, in_maps, list(range(2 * B)))
    return _merge(res.results, B, T)
